# revision 16
# baseline (speedup 1.0000x reference)
"""CrossAttention kernel for 8 TRN2 NeuronCores.

Sharding: core c handles batch b = c//2 and query-half hf = c%2 (1024 of the
2048 query tokens). Keys come from pos_emb (batch-independent) and values from
context[b]; both are computed redundantly where needed so no collectives are
required — every core writes a disjoint [1024, 512] slice of the output.

Host-side prep folds the layernorm affine params into the projection weights:
  q = ((x-mu)*rstd) @ (ln_w[:,None]*Wq) + ln_b@Wq
so the device only computes the center/scale part of each layernorm.

Device pipeline (all fp32):
  1. LN center+scale token-major, PE-transpose to feature-major.
  2. Q/K projections emit Q^T/K^T (head-dim on partitions); V projection stays
     token-major and is stored per-head with a half-array of ones columns:
     even head h: [v_h | 1...], odd: [1... | v_h].
  3. sim^T = K^T.T @ Q^T per head, two heads packed per 128-partition chunk
     (K=64 row-packed matmuls at base partitions 0 and 64).
  4. exp via one ACT activation per 4-bank PSUM group (scale=1/8 fused).
  5. AV matmul per head: lhsT = v_ext -> psum rows give unnormalized attn
     output on one partition half and the softmax denominator replicated on
     the other half. Reciprocal + partition-shift DMA + multiply normalizes.
  6. Final projection uses O^T chunks as stationary -> token-major output.
"""

import numpy as np

import concourse.bass as bass
import concourse.mybir as mybir
import concourse.tile as tile
from concourse import bacc
from concourse.bass import ts
from concourse.bass_utils import run_bass_kernel_spmd
from concourse.masks import make_identity

B, N, M, F, H, D = 4, 2048, 2048, 512, 8, 64
MID = H * D
EPS = 1e-5
NCORES = 8
NQ = N // 2  # query tokens per core
P = 128
FC = F // P  # feature chunks (4)
DC = MID // P  # output-dim chunks / head pairs (4)
MC = M // P  # key/value chunks (16)
SCALE = float(D) ** -0.5

FP32 = mybir.dt.float32
BF16 = mybir.dt.bfloat16
AF = mybir.ActivationFunctionType
ALU = mybir.AluOpType

NQB = 256  # query block for attention
G = 4  # m-chunks per exp group (4 psum banks)

_cache = {}


def _p_bcast(ap, p):
    """Broadcast a 1-D (free-only) AP across p partitions (stride-0)."""
    return bass.AP(tensor=ap.tensor, offset=ap.offset, ap=[[0, p], *ap.ap])


def _emit(tc, nc, t):
    v = nc.vector
    sc = nc.scalar
    te = nc.tensor

    consts_cm = tc.tile_pool(name="consts", bufs=1)
    consts = consts_cm.__enter__()

    # Weights, feature-major chunked: [P, FC, out]
    wq_sb = consts.tile([P, FC, MID], FP32)
    wk_sb = consts.tile([P, FC, MID], FP32)
    wv_sb = consts.tile([P, FC, MID], FP32)
    wo_sb = consts.tile([P, DC, F], FP32)
    nc.sync.dma_start(wq_sb, t["wq"].ap().rearrange("(c p) n -> p c n", p=P))
    nc.sync.dma_start(wk_sb, t["wk"].ap().rearrange("(c p) n -> p c n", p=P))
    nc.sync.dma_start(wv_sb, t["wv"].ap().rearrange("(c p) n -> p c n", p=P))
    nc.sync.dma_start(wo_sb, t["wo"].ap().rearrange("(c p) n -> p c n", p=P))

    # Projection biases: c2q/c2k as per-partition columns [P, DC]
    c2q_sb = consts.tile([P, DC], FP32)
    c2k_sb = consts.tile([P, DC], FP32)
    nc.sync.dma_start(c2q_sb, t["c2q"].ap().rearrange("(c p) -> p c", p=P))
    nc.sync.dma_start(c2k_sb, t["c2k"].ap().rearrange("(c p) -> p c", p=P))
    # c2v / bout broadcast across partitions as [P, 512] rows
    c2v_b = consts.tile([P, MID], FP32)
    bout_b = consts.tile([P, F], FP32)
    nc.sync.dma_start(c2v_b, _p_bcast(t["c2v"].ap(), P))
    nc.sync.dma_start(bout_b, _p_bcast(t["bout"].ap(), P))

    ident = consts.tile([P, P], FP32)
    make_identity(nc, ident)
    eps_sb = consts.tile([P, 1], FP32)
    v.memset(eps_sb, EPS)

    # Persistent activation tensors
    KT = consts.tile([P, DC, M], FP32)  # K^T  32KB/partition
    QT = consts.tile([P, DC, NQ], FP32)  # Q^T  16KB/partition
    vext = consts.tile([P, MC, H, P], BF16)  # per-head [v|1] / [1|v]  32KB/partition
    # ones halves: even heads cols 64:128, odd heads cols 0:64
    nc.gpsimd.memset(vext[:, :, 0::2, 64:128], 1.0)
    nc.gpsimd.memset(vext[:, :, 1::2, 0:64], 1.0)

    # ---------------- Phase 1: LN + transpose + projections ----------------
    with (
        tc.tile_pool(name="src", bufs=2) as srcp,
        tc.tile_pool(name="stats", bufs=2) as statsp,
        tc.tile_pool(name="actT", bufs=3) as actTp,
        tc.tile_pool(name="tpsum", bufs=4, space="PSUM") as tpsum,
        tc.tile_pool(name="ppsum", bufs=3, space="PSUM") as ppsum,
    ):

        def ln_transpose(t_src, toks, chunk_cb):
            """Load [toks,F] in 1024-token segments, layernorm center+scale,
            and for each 512-token chunk hand the feature-major transpose
            [P, FC, 512] to chunk_cb."""
            src_ap = t_src.ap().rearrange("(t p) f -> p t f", p=P)
            for seg in range(toks // 1024):
                T = 8
                src = srcp.tile([P, T, F], FP32, tag="src")
                nc.sync.dma_start(src, src_ap[:, ts(seg, T), :])
                stats = statsp.tile([P, T, 6], FP32, tag="stats")
                mv = statsp.tile([P, T, 2], FP32, tag="mv")
                rstd = statsp.tile([P, T], FP32, tag="rstd")
                for i in range(T):
                    v.bn_stats(stats[:, i, :], src[:, i, :])
                    v.bn_aggr(mv[:, i, :], stats[:, i, :])
                sc.activation(rstd, mv[:, :, 1], func=AF.Sqrt, bias=eps_sb, scale=1.0)
                v.reciprocal(rstd, rstd)
                for i in range(T):
                    v.tensor_scalar(
                        out=src[:, i, :],
                        in0=src[:, i, :],
                        scalar1=mv[:, i, 0:1],
                        scalar2=rstd[:, i : i + 1],
                        op0=ALU.subtract,
                        op1=ALU.mult,
                    )
                for c in range(T // 4):
                    zT = actTp.tile([P, FC, 512], FP32, tag="zT")
                    for tl in range(4):
                        i = c * 4 + tl
                        for fc in range(FC):
                            tp = tpsum.tile([P, P], FP32, tag="tp")
                            te.transpose(tp, src[:, i, ts(fc, P)], ident)
                            v.tensor_copy(out=zT[:, fc, ts(tl, P)], in_=tp)
                    chunk_cb(seg * 2 + c, zT)

        def k_chunk(c, zT):
            for dc in range(DC):
                ps = ppsum.tile([P, 512], FP32, tag="proj")
                for fc in range(FC):
                    te.matmul(
                        ps,
                        lhsT=wk_sb[:, fc, ts(dc, P)],
                        rhs=zT[:, fc, :],
                        start=(fc == 0),
                        stop=(fc == FC - 1),
                    )
                v.tensor_scalar_add(
                    out=KT[:, dc, ts(c, 512)], in0=ps, scalar1=c2k_sb[:, dc : dc + 1]
                )

        def q_chunk(c, zT):
            for dc in range(DC):
                ps = ppsum.tile([P, 512], FP32, tag="proj")
                for fc in range(FC):
                    te.matmul(
                        ps,
                        lhsT=wq_sb[:, fc, ts(dc, P)],
                        rhs=zT[:, fc, :],
                        start=(fc == 0),
                        stop=(fc == FC - 1),
                    )
                v.tensor_scalar_add(
                    out=QT[:, dc, ts(c, 512)], in0=ps, scalar1=c2q_sb[:, dc : dc + 1]
                )

        def v_chunk(c, zT):
            for mtl in range(4):
                mt = c * 4 + mtl
                ps = ppsum.tile([P, 512], FP32, tag="proj")
                for fc in range(FC):
                    te.matmul(
                        ps,
                        lhsT=zT[:, fc, ts(mtl, P)],
                        rhs=wv_sb[:, fc, :],
                        start=(fc == 0),
                        stop=(fc == FC - 1),
                    )
                psv = ps.rearrange("p (h d) -> p h d", h=H)
                cvv = c2v_b.rearrange("p (h d) -> p h d", h=H)
                v.tensor_tensor(
                    out=vext[:, mt, 0::2, 0:64],
                    in0=psv[:, 0::2, :],
                    in1=cvv[:, 0::2, :],
                    op=ALU.add,
                )
                v.tensor_tensor(
                    out=vext[:, mt, 1::2, 64:128],
                    in0=psv[:, 1::2, :],
                    in1=cvv[:, 1::2, :],
                    op=ALU.add,
                )

        ln_transpose(t["pe"], M, k_chunk)
        ln_transpose(t["xs"], NQ, q_chunk)
        ln_transpose(t["ctx"], M, v_chunk)

    # ---------------- Phase 2: attention + output projection ----------------
    out_t = t["out"].ap().rearrange("(t p) f -> t p f", p=P)
    with (
        tc.tile_pool(name="spsum", bufs=1, space="PSUM") as spsum,
        tc.tile_pool(name="apsum", bufs=1, space="PSUM") as apsum,
        tc.tile_pool(name="fpsum", bufs=2, space="PSUM") as fpsum,
        tc.tile_pool(name="et", bufs=3) as etp,
        tc.tile_pool(name="dr", bufs=2) as drp,
        tc.tile_pool(name="fo", bufs=2) as fop,
        tc.tile_pool(name="otb", bufs=2) as otbp,
    ):
        for nq in range(NQ // NQB):
            OTb = otbp.tile([P, DC, NQB], FP32, tag="otb")
            for dc in range(DC):
                avA = apsum.tile([P, NQB], FP32, tag="avA")
                avB = apsum.tile([P, NQB], FP32, tag="avB")
                for g in range(MC // G):
                    sp = spsum.tile([P, 2, G, NQB], FP32, tag="sp")
                    for mg in range(G):
                        mc = g * G + mg
                        te.matmul(
                            sp[:, 0, mg, :],
                            lhsT=KT[0:64, dc, ts(mc, P)],
                            rhs=QT[0:64, dc, ts(nq, NQB)],
                            start=True,
                            stop=True,
                        )
                        te.matmul(
                            sp[:, 1, mg, :],
                            lhsT=KT[64:128, dc, ts(mc, P)],
                            rhs=QT[64:128, dc, ts(nq, NQB)],
                            start=True,
                            stop=True,
                        )
                    et = etp.tile([P, 2, G, NQB], BF16, tag="et")
                    sc.activation(out=et, in_=sp, func=AF.Exp, scale=SCALE)
                    for mg in range(G):
                        mc = g * G + mg
                        for hh in range(2):
                            av = avA if hh == 0 else avB
                            te.matmul(
                                av,
                                lhsT=vext[:, mc, 2 * dc + hh, :],
                                rhs=et[:, hh, mg, :],
                                start=(mc == 0),
                                stop=(mc == MC - 1),
                                skip_group_check=True,
                            )
                # normalize: Õ on one partition half, Z replicated on the other
                for hh in range(2):
                    av = avA if hh == 0 else avB
                    par = hh * 64  # Õ partitions
                    zb = 64 - par  # Z partitions
                    rz = drp.tile([P, NQB], FP32, tag="rz")
                    v.reciprocal(rz[zb : zb + 64, :], av[zb : zb + 64, :])
                    zs = drp.tile([P, NQB], FP32, tag="zs")
                    nc.sync.dma_start(zs[par : par + 64, :], rz[zb : zb + 64, :])
                    v.tensor_mul(
                        out=OTb[par : par + 64, dc, :],
                        in0=av[par : par + 64, :],
                        in1=zs[par : par + 64, :],
                    )
            for ncl in range(NQB // P):
                nchunk = nq * (NQB // P) + ncl
                fp = fpsum.tile([P, F], FP32, tag="fp")
                for ko in range(DC):
                    te.matmul(
                        fp,
                        lhsT=OTb[:, ko, ts(ncl, P)],
                        rhs=wo_sb[:, ko, :],
                        start=(ko == 0),
                        stop=(ko == DC - 1),
                    )
                fo = fop.tile([P, F], FP32, tag="fo")
                v.tensor_tensor(out=fo, in0=fp, in1=bout_b, op=ALU.add)
                nc.sync.dma_start(out_t[nchunk], fo)

    consts_cm.__exit__(None, None, None)


def build():
    if "nc" in _cache:
        return _cache["nc"]
    nc = bacc.Bacc("TRN2", debug=False, num_devices=NCORES)
    t = {}
    t["xs"] = nc.dram_tensor("xs", [NQ, F], FP32, kind="ExternalInput")
    t["ctx"] = nc.dram_tensor("ctx", [M, F], FP32, kind="ExternalInput")
    t["pe"] = nc.dram_tensor("pe", [M, F], FP32, kind="ExternalInput")
    t["wq"] = nc.dram_tensor("wq", [F, MID], FP32, kind="ExternalInput")
    t["wk"] = nc.dram_tensor("wk", [F, MID], FP32, kind="ExternalInput")
    t["wv"] = nc.dram_tensor("wv", [F, MID], FP32, kind="ExternalInput")
    t["wo"] = nc.dram_tensor("wo", [MID, F], FP32, kind="ExternalInput")
    t["c2q"] = nc.dram_tensor("c2q", [MID], FP32, kind="ExternalInput")
    t["c2k"] = nc.dram_tensor("c2k", [MID], FP32, kind="ExternalInput")
    t["c2v"] = nc.dram_tensor("c2v", [MID], FP32, kind="ExternalInput")
    t["bout"] = nc.dram_tensor("bout", [F], FP32, kind="ExternalInput")
    t["out"] = nc.dram_tensor("out", [NQ, F], FP32, kind="ExternalOutput")
    with tile.TileContext(nc) as tc:
        _emit(tc, nc, t)
    nc.compile()
    _cache["nc"] = nc
    return nc


def make_in_maps(inputs):
    f32 = lambda a: np.ascontiguousarray(np.asarray(a, dtype=np.float32))
    x = f32(inputs["x"])
    context = f32(inputs["context"])
    pos_emb = f32(inputs["pos_emb"])
    ln_w, ln_b = f32(inputs["ln_w"]), f32(inputs["ln_b"])
    lnc_w, lnc_b = f32(inputs["lnc_w"]), f32(inputs["lnc_b"])
    Wq, Wk, Wv = f32(inputs["Wq"]), f32(inputs["Wk"]), f32(inputs["Wv"])
    Wout, bout = f32(inputs["Wout"]), f32(inputs["bout"])

    # fold LN affine into projections (host-side, weights only)
    wq_p = f32(ln_w[:, None] * Wq)
    wk_p = f32(ln_w[:, None] * Wk)
    wv_p = f32(lnc_w[:, None] * Wv)
    c2q = f32(ln_b @ Wq)
    c2k = f32(ln_b @ Wk)
    c2v = f32(lnc_b @ Wv)

    in_maps = []
    for c in range(NCORES):
        b, hf = divmod(c, 2)
        in_maps.append(
            {
                "xs": f32(x[b, hf * NQ : (hf + 1) * NQ]),
                "ctx": context[b],
                "pe": pos_emb,
                "wq": wq_p,
                "wk": wk_p,
                "wv": wv_p,
                "wo": Wout,
                "c2q": c2q,
                "c2k": c2k,
                "c2v": c2v,
                "bout": bout,
            }
        )
    return in_maps


def assemble(results):
    out = np.empty((B, N, F), np.float32)
    for c in range(NCORES):
        b, hf = divmod(c, 2)
        out[b, hf * NQ : (hf + 1) * NQ] = results[c]["out"]
    return out


def kernel(**inputs):
    nc = build()
    in_maps = make_in_maps(inputs)
    res = run_bass_kernel_spmd(nc, in_maps, core_ids=list(range(NCORES)))
    return assemble(res.results)


if __name__ == "__main__":
    rng = np.random.default_rng(0)
    ins = {
        "x": rng.standard_normal((B, N, F), dtype=np.float32),
        "context": rng.standard_normal((B, M, F), dtype=np.float32),
        "pos_emb": rng.standard_normal((M, F), dtype=np.float32) * 0.02,
        "ln_w": np.ones(F, np.float32),
        "ln_b": np.zeros(F, np.float32),
        "lnc_w": np.ones(F, np.float32),
        "lnc_b": np.zeros(F, np.float32),
        "Wq": rng.standard_normal((F, MID), dtype=np.float32) * 0.02,
        "Wk": rng.standard_normal((F, MID), dtype=np.float32) * 0.02,
        "Wv": rng.standard_normal((F, MID), dtype=np.float32) * 0.02,
        "Wout": rng.standard_normal((MID, F), dtype=np.float32) * 0.02,
        "bout": np.zeros(F, np.float32),
    }
    out = kernel(**ins)
    print(out.shape, out.dtype, np.abs(out).max())


# revision 19
# speedup vs baseline: 1.4064x; 1.4064x over previous
"""CrossAttention kernel for 8 TRN2 NeuronCores.

Sharding: core c handles batch b = c//2 and query-half hf = c%2 (1024 of the
2048 query tokens). Keys come from pos_emb (batch-independent) and values from
context[b]; both are computed redundantly where needed so no collectives are
required — every core writes a disjoint [1024, 512] slice of the output.

Host-side prep folds the layernorm affine params into the projection weights:
  q = ((x-mu)*rstd) @ (ln_w[:,None]*Wq) + ln_b@Wq
so the device only computes the center/scale part of each layernorm. Weights
are shipped bf16 (fp32 matmuls on TRN2 run two-pass LOW_HIGH at half rate).

Device pipeline:
  1. LN center+scale token-major (fp32 stats), writing bf16; PE-transpose to
     feature-major.
  2. Q/K projections emit Q^T/K^T bf16 (head-dim on partitions); V projection
     stays token-major, stored per-head with a half-array of ones columns:
     even head h: [v_h | 1...], odd: [1... | v_h].
  3. sim^T = K^T.T @ Q^T per head, two heads packed per 128-partition chunk
     (K=64 row-packed matmuls at base partitions 0 and 64), fp32 PSUM.
  4. exp via one ACT activation per 4-bank PSUM group (scale=1/8 fused),
     output bf16.
  5. AV matmul per head: lhsT = v_ext -> psum rows give unnormalized attn
     output on one partition half and the softmax denominator replicated on
     the other half. Fast reciprocal + partition-shift DMA + multiply
     normalizes into O^T (bf16).
  6. Final projection uses O^T chunks as stationary -> token-major output.
"""

import ml_dtypes
import numpy as np

import concourse.bass as bass
import concourse.mybir as mybir
import concourse.tile as tile
from concourse import bacc
from concourse.bass import ts
from concourse.bass_utils import run_bass_kernel_spmd
from concourse.masks import make_identity

B, N, M, F, H, D = 4, 2048, 2048, 512, 8, 64
MID = H * D
EPS = 1e-5
NCORES = 8
NQ = N // 2  # query tokens per core
P = 128
FC = F // P  # feature chunks (4)
DC = MID // P  # output-dim chunks / head pairs (4)
MC = M // P  # key/value chunks (16)
SCALE = float(D) ** -0.5

FP32 = mybir.dt.float32
BF16 = mybir.dt.bfloat16
AF = mybir.ActivationFunctionType
ALU = mybir.AluOpType

NQB = 512  # query block for attention
G = 2  # m-chunks per exp group (4 psum banks with 2 heads)

_cache = {}


def _p_bcast(ap, p):
    """Broadcast a 1-D (free-only) AP across p partitions (stride-0)."""
    return bass.AP(tensor=ap.tensor, offset=ap.offset, ap=[[0, p], *ap.ap])


def _emit(tc, nc, t):
    v = nc.vector
    sc = nc.scalar
    te = nc.tensor

    consts_cm = tc.tile_pool(name="consts", bufs=1)
    consts = consts_cm.__enter__()

    # Weights (bf16 from host), feature-major chunked: [P, FC, out]
    wq_sb = consts.tile([P, FC, MID], BF16)
    wk_sb = consts.tile([P, FC, MID], BF16)
    wv_sb = consts.tile([P, FC, MID], BF16)
    wo_sb = consts.tile([P, DC, F], BF16)
    nc.sync.dma_start(wq_sb, t["wq"].ap().rearrange("(c p) n -> p c n", p=P))
    nc.sync.dma_start(wk_sb, t["wk"].ap().rearrange("(c p) n -> p c n", p=P))
    nc.sync.dma_start(wv_sb, t["wv"].ap().rearrange("(c p) n -> p c n", p=P))
    nc.sync.dma_start(wo_sb, t["wo"].ap().rearrange("(c p) n -> p c n", p=P))

    # Projection biases: c2q/c2k as per-partition columns [P, DC] (fp32)
    c2q_sb = consts.tile([P, DC], FP32)
    c2k_sb = consts.tile([P, DC], FP32)
    nc.sync.dma_start(c2q_sb, t["c2q"].ap().rearrange("(c p) -> p c", p=P))
    nc.sync.dma_start(c2k_sb, t["c2k"].ap().rearrange("(c p) -> p c", p=P))
    # c2v / bout broadcast across partitions (fp32)
    c2v_b = consts.tile([P, MID], FP32)
    bout_b = consts.tile([P, F], FP32)
    nc.sync.dma_start(c2v_b, _p_bcast(t["c2v"].ap(), P))
    nc.sync.dma_start(bout_b, _p_bcast(t["bout"].ap(), P))

    ident = consts.tile([P, P], BF16)
    make_identity(nc, ident)
    eps_sb = consts.tile([P, 1], FP32)
    v.memset(eps_sb, EPS)

    # Persistent activation tensors
    KT = consts.tile([P, DC, M], BF16)  # K^T  16KB/partition
    QT = consts.tile([P, DC, NQ], BF16)  # Q^T  8KB/partition
    vext = consts.tile([P, MC, H, P], BF16)  # per-head [v|1] / [1|v]  32KB/part
    # ones halves: even heads cols 64:128, odd heads cols 0:64
    nc.gpsimd.memset(vext[:, :, 0::2, 64:128], 1.0)
    nc.gpsimd.memset(vext[:, :, 1::2, 0:64], 1.0)

    # ---------------- Phase 1: LN + transpose + projections ----------------
    with (
        tc.tile_pool(name="src", bufs=1) as srcp,
        tc.tile_pool(name="zln", bufs=2) as zlnp,
        tc.tile_pool(name="stats", bufs=2) as statsp,
        tc.tile_pool(name="actT", bufs=3) as actTp,
        tc.tile_pool(name="tpsum", bufs=4, space="PSUM") as tpsum,
        tc.tile_pool(name="ppsum", bufs=3, space="PSUM") as ppsum,
    ):

        def ln_transpose(t_src, toks, chunk_cb):
            """Load [toks,F] in 1024-token segments, layernorm center+scale
            (bf16 out), and for each 512-token chunk hand the feature-major
            transpose [P, FC, 512] to chunk_cb."""
            src_ap = t_src.ap().rearrange("(t p) f -> p t f", p=P)
            for seg in range(toks // 1024):
                T = 8
                src = srcp.tile([P, T, F], FP32, tag="src")
                nc.sync.dma_start(src, src_ap[:, ts(seg, T), :])
                zln = zlnp.tile([P, T, F], BF16, tag="zln")
                stats = statsp.tile([P, T, 6], FP32, tag="stats")
                mv = statsp.tile([P, T, 2], FP32, tag="mv")
                rstd = statsp.tile([P, T], FP32, tag="rstd")
                for i in range(T):
                    v.bn_stats(stats[:, i, :], src[:, i, :])
                    v.bn_aggr(mv[:, i, :], stats[:, i, :])
                sc.activation(rstd, mv[:, :, 1], func=AF.Sqrt, bias=eps_sb, scale=1.0)
                v.reciprocal(rstd, rstd)
                for i in range(T):
                    v.tensor_scalar(
                        out=zln[:, i, :],
                        in0=src[:, i, :],
                        scalar1=mv[:, i, 0:1],
                        scalar2=rstd[:, i : i + 1],
                        op0=ALU.subtract,
                        op1=ALU.mult,
                    )
                for c in range(T // 4):
                    zT = actTp.tile([P, FC, 512], BF16, tag="zT")
                    for tl in range(4):
                        i = c * 4 + tl
                        for fc in range(FC):
                            tp = tpsum.tile([P, P], BF16, tag="tp")
                            te.transpose(tp, zln[:, i, ts(fc, P)], ident)
                            v.tensor_copy(out=zT[:, fc, ts(tl, P)], in_=tp)
                    chunk_cb(seg * 2 + c, zT)

        def qk_chunk(W_sb, c2_sb, OutT):
            def cb(c, zT):
                for dc in range(DC):
                    ps = ppsum.tile([P, 512], FP32, tag="proj")
                    for fc in range(FC):
                        te.matmul(
                            ps,
                            lhsT=W_sb[:, fc, ts(dc, P)],
                            rhs=zT[:, fc, :],
                            start=(fc == 0),
                            stop=(fc == FC - 1),
                        )
                    v.tensor_scalar_add(
                        out=OutT[:, dc, ts(c, 512)], in0=ps, scalar1=c2_sb[:, dc : dc + 1]
                    )

            return cb

        def v_chunk(c, zT):
            for mtl in range(4):
                mt = c * 4 + mtl
                ps = ppsum.tile([P, 512], FP32, tag="proj")
                for fc in range(FC):
                    te.matmul(
                        ps,
                        lhsT=zT[:, fc, ts(mtl, P)],
                        rhs=wv_sb[:, fc, :],
                        start=(fc == 0),
                        stop=(fc == FC - 1),
                    )
                psv = ps.rearrange("p (h d) -> p h d", h=H)
                cvv = c2v_b.rearrange("p (h d) -> p h d", h=H)
                v.tensor_tensor(
                    out=vext[:, mt, 0::2, 0:64],
                    in0=psv[:, 0::2, :],
                    in1=cvv[:, 0::2, :],
                    op=ALU.add,
                )
                v.tensor_tensor(
                    out=vext[:, mt, 1::2, 64:128],
                    in0=psv[:, 1::2, :],
                    in1=cvv[:, 1::2, :],
                    op=ALU.add,
                )

        ln_transpose(t["pe"], M, qk_chunk(wk_sb, c2k_sb, KT))
        ln_transpose(t["xs"], NQ, qk_chunk(wq_sb, c2q_sb, QT))
        ln_transpose(t["ctx"], M, v_chunk)

    # ---------------- Phase 2: attention + output projection ----------------
    out_t = t["out"].ap().rearrange("(t p) f -> t p f", p=P)
    with (
        tc.tile_pool(name="spsum", bufs=1, space="PSUM") as spsum,
        tc.tile_pool(name="apsum", bufs=1, space="PSUM") as apsum,
        tc.tile_pool(name="fpsum", bufs=2, space="PSUM") as fpsum,
        tc.tile_pool(name="et", bufs=6) as etp,
        tc.tile_pool(name="dr", bufs=2) as drp,
        tc.tile_pool(name="fo", bufs=2) as fop,
        tc.tile_pool(name="otb", bufs=2) as otbp,
    ):
        for nq in range(NQ // NQB):
            OTb = otbp.tile([P, DC, NQB], BF16, tag="otb")
            for dc in range(DC):
                avA = apsum.tile([P, NQB], FP32, tag="avA")
                avB = apsum.tile([P, NQB], FP32, tag="avB")
                for g in range(MC // G):
                    sp = spsum.tile([P, 2, G, NQB], FP32, tag="sp")
                    for mg in range(G):
                        mc = g * G + mg
                        te.matmul(
                            sp[:, 0, mg, :],
                            lhsT=KT[0:64, dc, ts(mc, P)],
                            rhs=QT[0:64, dc, ts(nq, NQB)],
                            start=True,
                            stop=True,
                        )
                        te.matmul(
                            sp[:, 1, mg, :],
                            lhsT=KT[64:128, dc, ts(mc, P)],
                            rhs=QT[64:128, dc, ts(nq, NQB)],
                            start=True,
                            stop=True,
                        )
                    et = etp.tile([P, 2, G, NQB], BF16, tag="et")
                    sc.activation(out=et, in_=sp, func=AF.Exp, scale=SCALE)
                    for mg in range(G):
                        mc = g * G + mg
                        for hh in range(2):
                            av = avA if hh == 0 else avB
                            te.matmul(
                                av,
                                lhsT=vext[:, mc, 2 * dc + hh, :],
                                rhs=et[:, hh, mg, :],
                                start=(mc == 0),
                                stop=(mc == MC - 1),
                                skip_group_check=True,
                            )
                # normalize: Õ on one partition half, Z replicated on the other
                for hh in range(2):
                    av = avA if hh == 0 else avB
                    par = hh * 64  # Õ partitions
                    zb = 64 - par  # Z partitions
                    rz = drp.tile([P, NQB], FP32, tag="rz")
                    v.reciprocal(rz[zb : zb + 64, :], av[zb : zb + 64, :])
                    zs = drp.tile([P, NQB], FP32, tag="zs")
                    nc.sync.dma_start(zs[par : par + 64, :], rz[zb : zb + 64, :])
                    v.tensor_mul(
                        out=OTb[par : par + 64, dc, :],
                        in0=av[par : par + 64, :],
                        in1=zs[par : par + 64, :],
                    )
            for ncl in range(NQB // P):
                nchunk = nq * (NQB // P) + ncl
                fp = fpsum.tile([P, F], FP32, tag="fp")
                for ko in range(DC):
                    te.matmul(
                        fp,
                        lhsT=OTb[:, ko, ts(ncl, P)],
                        rhs=wo_sb[:, ko, :],
                        start=(ko == 0),
                        stop=(ko == DC - 1),
                    )
                fo = fop.tile([P, F], FP32, tag="fo")
                v.tensor_tensor(out=fo, in0=fp, in1=bout_b, op=ALU.add)
                nc.sync.dma_start(out_t[nchunk], fo)

    consts_cm.__exit__(None, None, None)


def build():
    if "nc" in _cache:
        return _cache["nc"]
    nc = bacc.Bacc("TRN2", debug=False, num_devices=NCORES)
    t = {}
    t["xs"] = nc.dram_tensor("xs", [NQ, F], FP32, kind="ExternalInput")
    t["ctx"] = nc.dram_tensor("ctx", [M, F], FP32, kind="ExternalInput")
    t["pe"] = nc.dram_tensor("pe", [M, F], FP32, kind="ExternalInput")
    t["wq"] = nc.dram_tensor("wq", [F, MID], BF16, kind="ExternalInput")
    t["wk"] = nc.dram_tensor("wk", [F, MID], BF16, kind="ExternalInput")
    t["wv"] = nc.dram_tensor("wv", [F, MID], BF16, kind="ExternalInput")
    t["wo"] = nc.dram_tensor("wo", [MID, F], BF16, kind="ExternalInput")
    t["c2q"] = nc.dram_tensor("c2q", [MID], FP32, kind="ExternalInput")
    t["c2k"] = nc.dram_tensor("c2k", [MID], FP32, kind="ExternalInput")
    t["c2v"] = nc.dram_tensor("c2v", [MID], FP32, kind="ExternalInput")
    t["bout"] = nc.dram_tensor("bout", [F], FP32, kind="ExternalInput")
    t["out"] = nc.dram_tensor("out", [NQ, F], FP32, kind="ExternalOutput")
    with tile.TileContext(nc) as tc:
        _emit(tc, nc, t)
    nc.compile()
    _cache["nc"] = nc
    return nc


def make_in_maps(inputs):
    f32 = lambda a: np.ascontiguousarray(np.asarray(a, dtype=np.float32))
    bf16 = lambda a: np.ascontiguousarray(np.asarray(a, dtype=np.float32)).astype(
        ml_dtypes.bfloat16
    )
    x = f32(inputs["x"])
    context = f32(inputs["context"])
    pos_emb = f32(inputs["pos_emb"])
    ln_w, ln_b = f32(inputs["ln_w"]), f32(inputs["ln_b"])
    lnc_w, lnc_b = f32(inputs["lnc_w"]), f32(inputs["lnc_b"])
    Wq, Wk, Wv = f32(inputs["Wq"]), f32(inputs["Wk"]), f32(inputs["Wv"])
    Wout, bout = f32(inputs["Wout"]), f32(inputs["bout"])

    # fold LN affine into projections (host-side, weights only)
    wq_p = bf16(ln_w[:, None] * Wq)
    wk_p = bf16(ln_w[:, None] * Wk)
    wv_p = bf16(lnc_w[:, None] * Wv)
    c2q = f32(ln_b @ Wq)
    c2k = f32(ln_b @ Wk)
    c2v = f32(lnc_b @ Wv)

    in_maps = []
    for c in range(NCORES):
        b, hf = divmod(c, 2)
        in_maps.append(
            {
                "xs": f32(x[b, hf * NQ : (hf + 1) * NQ]),
                "ctx": context[b],
                "pe": pos_emb,
                "wq": wq_p,
                "wk": wk_p,
                "wv": wv_p,
                "wo": bf16(Wout),
                "c2q": c2q,
                "c2k": c2k,
                "c2v": c2v,
                "bout": bout,
            }
        )
    return in_maps


def assemble(results):
    out = np.empty((B, N, F), np.float32)
    for c in range(NCORES):
        b, hf = divmod(c, 2)
        out[b, hf * NQ : (hf + 1) * NQ] = results[c]["out"]
    return out


def kernel(**inputs):
    nc = build()
    in_maps = make_in_maps(inputs)
    res = run_bass_kernel_spmd(nc, in_maps, core_ids=list(range(NCORES)))
    return assemble(res.results)


# revision 26
# speedup vs baseline: 1.6302x; 1.1591x over previous
"""CrossAttention kernel for 8 TRN2 NeuronCores.

Sharding: core c handles batch b = c//2 and query-half hf = c%2 (1024 of the
2048 query tokens). Keys come from pos_emb (batch-independent) and values from
context[b]; both are computed redundantly where needed so no collectives are
required — every core writes a disjoint [1024, 512] slice of the output.

Host-side prep folds the layernorm affine params into the projection weights:
  q = ((x-mu)*rstd) @ (ln_w[:,None]*Wq) + ln_b@Wq
so the device only computes the center/scale part of each layernorm. Weights
are shipped bf16 (fp32 matmuls on TRN2 run two-pass LOW_HIGH at half rate).

Device pipeline:
  1. LN center+scale token-major (fp32 stats), writing bf16; PE-transpose to
     feature-major.
  2. Q/K projections emit Q^T/K^T bf16 (head-dim on partitions); V projection
     stays token-major, stored per-head with a half-array of ones columns:
     even head h: [v_h | 1...], odd: [1... | v_h].
  3. sim^T = K^T.T @ Q^T per head, two heads packed per 128-partition chunk
     (K=64 row-packed matmuls at base partitions 0 and 64), fp32 PSUM.
  4. exp via one ACT activation per 4-bank PSUM group (scale=1/8 fused),
     output bf16.
  5. AV matmul per head: lhsT = v_ext -> psum rows give unnormalized attn
     output on one partition half and the softmax denominator replicated on
     the other half. Fast reciprocal + partition-shift DMA + multiply
     normalizes into O^T (bf16).
  6. Final projection uses O^T chunks as stationary -> token-major output.
"""

import ml_dtypes
import numpy as np

import concourse.bass as bass
import concourse.mybir as mybir
import concourse.tile as tile
from concourse import bacc
from concourse.bass import ts
from concourse.bass_utils import run_bass_kernel_spmd
from concourse.masks import make_identity

B, N, M, F, H, D = 4, 2048, 2048, 512, 8, 64
MID = H * D
EPS = 1e-5
NCORES = 8
NQ = N // 2  # query tokens per core
P = 128
FC = F // P  # feature chunks (4)
DC = MID // P  # output-dim chunks / head pairs (4)
MC = M // P  # key/value chunks (16)
SCALE = float(D) ** -0.5

FP32 = mybir.dt.float32
BF16 = mybir.dt.bfloat16
AF = mybir.ActivationFunctionType
ALU = mybir.AluOpType

NQB = 512  # query block for attention

_cache = {}


def _p_bcast(ap, p):
    """Broadcast a 1-D (free-only) AP across p partitions (stride-0)."""
    return bass.AP(tensor=ap.tensor, offset=ap.offset, ap=[[0, p], *ap.ap])


def _emit(tc, nc, t):
    v = nc.vector
    sc = nc.scalar
    te = nc.tensor

    consts_cm = tc.tile_pool(name="consts", bufs=1)
    consts = consts_cm.__enter__()

    # Weights (bf16 from host), feature-major chunked: [P, FC, out]
    wq_sb = consts.tile([P, FC, MID], BF16)
    wk_sb = consts.tile([P, FC, MID], BF16)
    wv_sb = consts.tile([P, FC, MID], BF16)
    wo_sb = consts.tile([P, DC, F], BF16)
    nc.sync.dma_start(wq_sb, t["wq"].ap().rearrange("(c p) n -> p c n", p=P))
    nc.sync.dma_start(wk_sb, t["wk"].ap().rearrange("(c p) n -> p c n", p=P))
    nc.sync.dma_start(wv_sb, t["wv"].ap().rearrange("(c p) n -> p c n", p=P))
    nc.sync.dma_start(wo_sb, t["wo"].ap().rearrange("(c p) n -> p c n", p=P))

    # Projection biases: c2q/c2k as per-partition columns [P, DC] (fp32)
    c2q_sb = consts.tile([P, DC], FP32)
    c2k_sb = consts.tile([P, DC], FP32)
    nc.sync.dma_start(c2q_sb, t["c2q"].ap().rearrange("(c p) -> p c", p=P))
    nc.sync.dma_start(c2k_sb, t["c2k"].ap().rearrange("(c p) -> p c", p=P))
    # c2v / bout broadcast across partitions (fp32)
    c2v_b = consts.tile([P, MID], FP32)
    bout_b = consts.tile([P, F], FP32)
    nc.sync.dma_start(c2v_b, _p_bcast(t["c2v"].ap(), P))
    nc.sync.dma_start(bout_b, _p_bcast(t["bout"].ap(), P))

    ident = consts.tile([P, P], BF16)
    make_identity(nc, ident)
    eps_sb = consts.tile([P, 1], FP32)
    v.memset(eps_sb, EPS)

    # Persistent activation tensors
    KT = consts.tile([P, DC, M], BF16)  # K^T  16KB/partition
    QT = consts.tile([P, DC, NQ], BF16)  # Q^T  8KB/partition
    vext = consts.tile([P, MC, H, P], BF16)  # per-head [v|1] / [1|v]  32KB/part
    # ones halves: even heads cols 64:128, odd heads cols 0:64
    nc.gpsimd.memset(vext[:, :, 0::2, 64:128], 1.0)
    nc.gpsimd.memset(vext[:, :, 1::2, 0:64], 1.0)

    # ---------------- Phase 1: LN + transpose + projections ----------------
    with (
        tc.tile_pool(name="src", bufs=2) as srcp,
        tc.tile_pool(name="zln", bufs=2) as zlnp,
        tc.tile_pool(name="stats", bufs=2) as statsp,
        tc.tile_pool(name="actT", bufs=3) as actTp,
        tc.tile_pool(name="tpsum", bufs=4, space="PSUM") as tpsum,
        tc.tile_pool(name="ppsum", bufs=3, space="PSUM") as ppsum,
    ):

        def ln_transpose(t_src, toks, chunk_cb):
            """Load [toks,F] in 1024-token segments, layernorm center+scale
            (bf16 out), and for each 512-token chunk hand the feature-major
            transpose [P, FC, 512] to chunk_cb."""
            src_ap = t_src.ap().rearrange("(t p) f -> p t f", p=P)
            for seg in range(toks // 1024):
                T = 8
                src = srcp.tile([P, T, F], FP32, tag="src")
                nc.sync.dma_start(src, src_ap[:, ts(seg, T), :])
                zln = zlnp.tile([P, T, F], BF16, tag="zln")
                stats = statsp.tile([P, T, 6], FP32, tag="stats")
                mv = statsp.tile([P, T, 2], FP32, tag="mv")
                rstd = statsp.tile([P, T], FP32, tag="rstd")
                for i in range(T):
                    v.bn_stats(stats[:, i, :], src[:, i, :])
                    v.bn_aggr(mv[:, i, :], stats[:, i, :])
                sc.activation(rstd, mv[:, :, 1], func=AF.Sqrt, bias=eps_sb, scale=1.0)
                v.reciprocal(rstd, rstd)
                for i in range(T):
                    v.tensor_scalar(
                        out=zln[:, i, :],
                        in0=src[:, i, :],
                        scalar1=mv[:, i, 0:1],
                        scalar2=rstd[:, i : i + 1],
                        op0=ALU.subtract,
                        op1=ALU.mult,
                    )
                for c in range(T // 4):
                    zT = actTp.tile([P, FC, 512], BF16, tag="zT")
                    for tl in range(4):
                        i = c * 4 + tl
                        tp = tpsum.tile([P, FC, P], BF16, tag="tp")
                        for fc in range(FC):
                            te.transpose(tp[:, fc, :], zln[:, i, ts(fc, P)], ident)
                        # one batched PSUM->SBUF copy per token tile
                        v.tensor_copy(out=zT[:, :, ts(tl, P)], in_=tp)
                    chunk_cb(seg * 2 + c, zT)

        def qk_chunk(W_sb, c2_sb, OutT):
            def cb(c, zT):
                for dc in range(DC):
                    ps = ppsum.tile([P, 512], FP32, tag="proj")
                    for fc in range(FC):
                        te.matmul(
                            ps,
                            lhsT=W_sb[:, fc, ts(dc, P)],
                            rhs=zT[:, fc, :],
                            start=(fc == 0),
                            stop=(fc == FC - 1),
                        )
                    v.tensor_scalar_add(
                        out=OutT[:, dc, ts(c, 512)], in0=ps, scalar1=c2_sb[:, dc : dc + 1]
                    )

            return cb

        def v_chunk(c, zT):
            for mtl in range(4):
                mt = c * 4 + mtl
                ps = ppsum.tile([P, 512], FP32, tag="proj")
                for fc in range(FC):
                    te.matmul(
                        ps,
                        lhsT=zT[:, fc, ts(mtl, P)],
                        rhs=wv_sb[:, fc, :],
                        start=(fc == 0),
                        stop=(fc == FC - 1),
                    )
                psv = ps.rearrange("p (h d) -> p h d", h=H)
                cvv = c2v_b.rearrange("p (h d) -> p h d", h=H)
                v.tensor_tensor(
                    out=vext[:, mt, 0::2, 0:64],
                    in0=psv[:, 0::2, :],
                    in1=cvv[:, 0::2, :],
                    op=ALU.add,
                )
                v.tensor_tensor(
                    out=vext[:, mt, 1::2, 64:128],
                    in0=psv[:, 1::2, :],
                    in1=cvv[:, 1::2, :],
                    op=ALU.add,
                )

        # x last: attention for query block 0 can start while later x segments
        # and nothing else remain in phase 1
        ln_transpose(t["pe"], M, qk_chunk(wk_sb, c2k_sb, KT))
        ln_transpose(t["ctx"], M, v_chunk)
        ln_transpose(t["xs"], NQ, qk_chunk(wq_sb, c2q_sb, QT))

    # ---------------- Phase 2: attention + output projection ----------------
    out_t = t["out"].ap().rearrange("(t p) f -> t p f", p=P)
    with (
        tc.tile_pool(name="spsum", bufs=2, space="PSUM") as spsum,
        tc.tile_pool(name="apsum", bufs=1, space="PSUM") as apsum,
        tc.tile_pool(name="fpsum", bufs=1, space="PSUM") as fpsum,
        tc.tile_pool(name="et", bufs=5) as etp,
        tc.tile_pool(name="dr", bufs=2) as drp,
        tc.tile_pool(name="fo", bufs=2) as fop,
        tc.tile_pool(name="otb", bufs=2) as otbp,
    ):
        for nq in range(NQ // NQB):
            OTb = otbp.tile([P, DC, NQB], BF16, tag="otb")
            for dc in range(DC):
                avA = apsum.tile([P, NQB], FP32, tag="avA")
                avB = apsum.tile([P, NQB], FP32, tag="avB")
                for mc in range(MC):
                    sp = spsum.tile([P, 2, NQB], FP32, tag="sp")
                    te.matmul(
                        sp[:, 0, :],
                        lhsT=KT[0:64, dc, ts(mc, P)],
                        rhs=QT[0:64, dc, ts(nq, NQB)],
                        start=True,
                        stop=True,
                    )
                    te.matmul(
                        sp[:, 1, :],
                        lhsT=KT[64:128, dc, ts(mc, P)],
                        rhs=QT[64:128, dc, ts(nq, NQB)],
                        start=True,
                        stop=True,
                    )
                    et = etp.tile([P, 2, NQB], BF16, tag="et")
                    sc.activation(out=et, in_=sp, func=AF.Exp, scale=SCALE)
                    for hh in range(2):
                        av = avA if hh == 0 else avB
                        te.matmul(
                            av,
                            lhsT=vext[:, mc, 2 * dc + hh, :],
                            rhs=et[:, hh, :],
                            start=(mc == 0),
                            stop=(mc == MC - 1),
                            skip_group_check=True,
                        )
                # normalize: Õ on one partition half, Z replicated on the other
                for hh in range(2):
                    av = avA if hh == 0 else avB
                    par = hh * 64  # Õ partitions
                    zb = 64 - par  # Z partitions
                    rz = drp.tile([P, NQB], FP32, tag="rz")
                    v.reciprocal(rz[zb : zb + 64, :], av[zb : zb + 64, :])
                    zs = drp.tile([P, NQB], FP32, tag="zs")
                    nc.sync.dma_start(zs[par : par + 64, :], rz[zb : zb + 64, :])
                    v.tensor_mul(
                        out=OTb[par : par + 64, dc, :],
                        in0=av[par : par + 64, :],
                        in1=zs[par : par + 64, :],
                    )
            for ncl in range(NQB // P):
                nchunk = nq * (NQB // P) + ncl
                fp = fpsum.tile([P, F], FP32, tag="fp")
                for ko in range(DC):
                    te.matmul(
                        fp,
                        lhsT=OTb[:, ko, ts(ncl, P)],
                        rhs=wo_sb[:, ko, :],
                        start=(ko == 0),
                        stop=(ko == DC - 1),
                    )
                fo = fop.tile([P, F], FP32, tag="fo")
                v.tensor_tensor(out=fo, in0=fp, in1=bout_b, op=ALU.add)
                nc.sync.dma_start(out_t[nchunk], fo)

    consts_cm.__exit__(None, None, None)


def build():
    if "nc" in _cache:
        return _cache["nc"]
    nc = bacc.Bacc("TRN2", debug=False, num_devices=NCORES)
    t = {}
    t["xs"] = nc.dram_tensor("xs", [NQ, F], FP32, kind="ExternalInput")
    t["ctx"] = nc.dram_tensor("ctx", [M, F], FP32, kind="ExternalInput")
    t["pe"] = nc.dram_tensor("pe", [M, F], FP32, kind="ExternalInput")
    t["wq"] = nc.dram_tensor("wq", [F, MID], BF16, kind="ExternalInput")
    t["wk"] = nc.dram_tensor("wk", [F, MID], BF16, kind="ExternalInput")
    t["wv"] = nc.dram_tensor("wv", [F, MID], BF16, kind="ExternalInput")
    t["wo"] = nc.dram_tensor("wo", [MID, F], BF16, kind="ExternalInput")
    t["c2q"] = nc.dram_tensor("c2q", [MID], FP32, kind="ExternalInput")
    t["c2k"] = nc.dram_tensor("c2k", [MID], FP32, kind="ExternalInput")
    t["c2v"] = nc.dram_tensor("c2v", [MID], FP32, kind="ExternalInput")
    t["bout"] = nc.dram_tensor("bout", [F], FP32, kind="ExternalInput")
    t["out"] = nc.dram_tensor("out", [NQ, F], FP32, kind="ExternalOutput")
    with tile.TileContext(nc) as tc:
        _emit(tc, nc, t)
    nc.compile()
    _cache["nc"] = nc
    return nc


def make_in_maps(inputs):
    f32 = lambda a: np.ascontiguousarray(np.asarray(a, dtype=np.float32))
    bf16 = lambda a: np.ascontiguousarray(np.asarray(a, dtype=np.float32)).astype(
        ml_dtypes.bfloat16
    )
    x = f32(inputs["x"])
    context = f32(inputs["context"])
    pos_emb = f32(inputs["pos_emb"])
    ln_w, ln_b = f32(inputs["ln_w"]), f32(inputs["ln_b"])
    lnc_w, lnc_b = f32(inputs["lnc_w"]), f32(inputs["lnc_b"])
    Wq, Wk, Wv = f32(inputs["Wq"]), f32(inputs["Wk"]), f32(inputs["Wv"])
    Wout, bout = f32(inputs["Wout"]), f32(inputs["bout"])

    # fold LN affine into projections (host-side, weights only)
    wq_p = bf16(ln_w[:, None] * Wq)
    wk_p = bf16(ln_w[:, None] * Wk)
    wv_p = bf16(lnc_w[:, None] * Wv)
    c2q = f32(ln_b @ Wq)
    c2k = f32(ln_b @ Wk)
    c2v = f32(lnc_b @ Wv)

    in_maps = []
    for c in range(NCORES):
        b, hf = divmod(c, 2)
        in_maps.append(
            {
                "xs": f32(x[b, hf * NQ : (hf + 1) * NQ]),
                "ctx": context[b],
                "pe": pos_emb,
                "wq": wq_p,
                "wk": wk_p,
                "wv": wv_p,
                "wo": bf16(Wout),
                "c2q": c2q,
                "c2k": c2k,
                "c2v": c2v,
                "bout": bout,
            }
        )
    return in_maps


def assemble(results):
    out = np.empty((B, N, F), np.float32)
    for c in range(NCORES):
        b, hf = divmod(c, 2)
        out[b, hf * NQ : (hf + 1) * NQ] = results[c]["out"]
    return out


def kernel(**inputs):
    nc = build()
    in_maps = make_in_maps(inputs)
    res = run_bass_kernel_spmd(nc, in_maps, core_ids=list(range(NCORES)))
    return assemble(res.results)
